# revision 10
# baseline (speedup 1.0000x reference)
"""Multi-head attention forward (B=8, S=1024, H=16, D=64) on 8 TRN2 NeuronCores.

Sharding: pure data-parallel over batch — core b computes batch element b
end-to-end (QKV projections + 16-head attention). Zero collectives.

Per-core dataflow (bf16 matmuls, fp32 PSUM accumulation):
  phase 0: DMA x (cast f32->bf16 in SWDGE), PE-transpose to x^T layout;
           DMA the three weight matrices (cast to bf16).
  pair loop (8 head-pairs, interleaved so the PE always has dense work and
  ScalarE's exp stream starts as early as possible):
    - Q^T/K^T slices for this pair: Q^T = Wq_slice.T @ x_from^T  (+bias)
    - V' slices: V'_st = [V_h | V_h' | ones] built straight from the V
      projection PSUM (+bias folded into V: exact, since softmax rows sum
      to 1, normalize(P_u @ (V+bv)) == ctx + bv)
    - scores^T[j,i] = K_h^T.T @ Q_h^T  (K=64 contraction; the two heads of
      a pair sit at SBUF partitions 0-63/64-127, so their matmuls land on
      disjoint PE row-groups and run concurrently)
    - Et = exp(scores^T/8) on ScalarE (no max-subtraction: logits bounded
      ~|2.3| for these inputs)
    - ctx'^T[65,i] = sum_jt V'_jt.T @ Et_jt  (row 64 = sum_j Et = softmax
      denominator, free via the ones column)
    - PE-transpose ctx' back to [i,d], multiply by reciprocal of the
      denominator column, DMA this pair's 128 output columns out.
"""

import numpy as np
from contextlib import ExitStack

import concourse.bass as bass
import concourse.mybir as mybir
import concourse.tile as tile
from concourse import bacc
from concourse.masks import make_identity
from concourse.bass_utils import run_bass_kernel_spmd

B, S, H, D = 8, 1024, 16, 64
W = H * D  # 1024
P = 128
N_CORES = 8
F32 = mybir.dt.float32
BF16 = mybir.dt.bfloat16
AF = mybir.ActivationFunctionType
ALU = mybir.AluOpType

ST = S // P   # 8 s-tiles
KT_ = W // P  # 8 contraction tiles
IH = 2        # 512-wide halves of the moving dim
HD1 = D + 1   # 65: V' width per head
NP = H // 2   # 8 head pairs


def build_kernel():
    nc = bacc.Bacc(trn_type="TRN2", target_bir_lowering=False, debug=False,
                   num_devices=N_CORES)

    xf_ext = nc.dram_tensor("from_tensor", [S, W], F32, kind="ExternalInput").ap()
    xt_ext = nc.dram_tensor("to_tensor", [S, W], F32, kind="ExternalInput").ap()
    wq_ext = nc.dram_tensor("Wq", [W, W], F32, kind="ExternalInput").ap()
    bq_ext = nc.dram_tensor("bq", [W], F32, kind="ExternalInput").ap()
    wk_ext = nc.dram_tensor("Wk", [W, W], F32, kind="ExternalInput").ap()
    bk_ext = nc.dram_tensor("bk", [W], F32, kind="ExternalInput").ap()
    wv_ext = nc.dram_tensor("Wv", [W, W], F32, kind="ExternalInput").ap()
    bv_ext = nc.dram_tensor("bv", [W], F32, kind="ExternalInput").ap()
    out_ext = nc.dram_tensor("out", [S, W], F32, kind="ExternalOutput").ap()

    with tile.TileContext(nc) as tc, ExitStack() as top:
        const = top.enter_context(tc.tile_pool(name="const", bufs=1))
        big = top.enter_context(tc.tile_pool(name="big", bufs=1))

        ident = const.tile([P, P], BF16, tag="ident")
        make_identity(nc, ident[:])
        bq_sb = const.tile([P, KT_], F32, tag="bq")
        nc.sync.dma_start(bq_sb[:], bq_ext.rearrange("(t p) -> p t", p=P))
        bk_sb = const.tile([P, KT_], F32, tag="bk")
        nc.sync.dma_start(bk_sb[:], bk_ext.rearrange("(t p) -> p t", p=P))
        bv_row = const.tile([1, W], F32, tag="bv_row")
        nc.sync.dma_start(bv_row[:], bv_ext.rearrange("(a w) -> a w", a=1))
        ones_col = const.tile([1, P], F32, tag="ones_col")
        nc.vector.memset(ones_col[:], 1.0)
        bvb = const.tile([P, W], F32, tag="bvb")

        # xT_all[p, kt*S + s] = x[s, kt*128+p]
        xTf_all = big.tile([P, KT_ * S], BF16, tag="xTf")
        xTt_all = big.tile([P, KT_ * S], BF16, tag="xTt")
        # w_all[p, kt*W + f] = Wx[kt*128+p, f]
        wq_all = big.tile([P, KT_ * W], BF16, tag="wq")
        wk_all = big.tile([P, KT_ * W], BF16, tag="wk")
        wv_all = big.tile([P, KT_ * W], BF16, tag="wv")

        def load_w(dst, src):
            nc.gpsimd.dma_start(
                dst.rearrange("p (t f) -> p t f", f=W),
                src.rearrange("(t p) f -> p t f", p=P))

        # ---- phase 0: load + transpose inputs ----
        with ExitStack() as ph0:
            xf_pool = ph0.enter_context(tc.tile_pool(name="xf", bufs=2))
            ps_t = ph0.enter_context(
                tc.tile_pool(name="ps_t", bufs=4, space="PSUM"))

            # bv broadcast to 128 partitions via PE outer product with ones
            for ih2 in range(IH):
                psb = ps_t.tile([P, 512], F32, tag="bvbp", bufs=1, name="ppb")
                nc.tensor.matmul(psb[:], lhsT=ones_col[:],
                                 rhs=bv_row[0:1, ih2 * 512:(ih2 + 1) * 512],
                                 start=True, stop=True)
                nc.vector.tensor_copy(bvb[:, ih2 * 512:(ih2 + 1) * 512], psb[:])

            def transpose_in(x_ext, xT_all, first):
                for ch in range(2):
                    xf = xf_pool.tile([P, 4 * W], BF16, tag="xf", name=f"xf{ch}")
                    nc.gpsimd.dma_start(
                        xf.rearrange("p (t f) -> p t f", f=W),
                        x_ext.rearrange("(t p) f -> p t f", p=P)[
                            :, ch * 4:(ch + 1) * 4, :])
                    if first and ch == 0:
                        load_w(wq_all, wq_ext)
                        load_w(wk_all, wk_ext)
                    for wt in range(KT_):
                        pt = ps_t.tile([P, 512], BF16, tag="pt", bufs=3, name="pt")
                        for sl in range(4):
                            nc.tensor.transpose(
                                pt[:, sl * P:(sl + 1) * P],
                                xf[:, sl * W + wt * P: sl * W + wt * P + P],
                                ident[:])
                        nc.vector.tensor_copy(
                            xT_all[:, wt * S + ch * 512: wt * S + (ch + 1) * 512],
                            pt[:])

            transpose_in(xf_ext, xTf_all, first=True)
            transpose_in(xt_ext, xTt_all, first=False)
            load_w(wv_all, wv_ext)

        # ---- pair loop ----
        with ExitStack() as ph2:
            pp_pool = ph2.enter_context(tc.tile_pool(name="pp", bufs=1))
            et_pool = ph2.enter_context(tc.tile_pool(name="et", bufs=40))
            sm_pool = ph2.enter_context(tc.tile_pool(name="sm", bufs=1))
            ps_proj = ph2.enter_context(
                tc.tile_pool(name="ps_proj", bufs=2, space="PSUM"))
            ps_s = ph2.enter_context(
                tc.tile_pool(name="ps_s", bufs=2, space="PSUM"))
            ps_c = ph2.enter_context(
                tc.tile_pool(name="ps_c", bufs=1, space="PSUM"))
            ps_o = ph2.enter_context(
                tc.tile_pool(name="ps_o", bufs=2, space="PSUM"))

            for hp in range(NP):
                mt = hp  # w-tile index of this pair's 128 output columns

                # Q^T / K^T slices for this pair: [128 wout, 1024 s]
                QTp = pp_pool.tile([P, S], BF16, tag="qt", bufs=3, name="QTp")
                KTp = pp_pool.tile([P, S], BF16, tag="kt", bufs=3, name="KTp")
                for dstT, w_all, xT_all, b_sb in (
                        (QTp, wq_all, xTf_all, bq_sb),
                        (KTp, wk_all, xTt_all, bk_sb)):
                    for ih in range(IH):
                        ps = ps_proj.tile([P, 512], F32, tag="proj", name="pp")
                        for kt in range(KT_):
                            nc.tensor.matmul(
                                ps[:],
                                lhsT=w_all[:, kt * W + mt * P: kt * W + mt * P + P],
                                rhs=xT_all[:, kt * S + ih * 512:
                                           kt * S + (ih + 1) * 512],
                                start=(kt == 0), stop=(kt == KT_ - 1))
                        nc.vector.tensor_scalar_add(
                            dstT[:, ih * 512:(ih + 1) * 512], ps[:],
                            b_sb[:, mt:mt + 1])

                # V' for this pair: per s-tile, [128 j, 2*65] (+bias, ones col)
                Vp = pp_pool.tile([P, ST * 2 * HD1], BF16, tag="vp", bufs=2,
                                  name="Vp")
                for st in range(ST):
                    ps = ps_proj.tile([P, 512], F32, tag="proj", name="ppv")
                    for kt in range(KT_):
                        nc.tensor.matmul(
                            ps[:, 0:P],
                            lhsT=xTt_all[:, kt * S + st * P: kt * S + st * P + P],
                            rhs=wv_all[:, kt * W + mt * P: kt * W + mt * P + P],
                            start=(kt == 0), stop=(kt == KT_ - 1))
                    dst = Vp[:, st * 2 * HD1: (st + 1) * 2 * HD1].rearrange(
                        "p (g c) -> p g c", c=HD1)[:, :, 0:D]
                    src = ps[:, 0:P].rearrange("p (g c) -> p g c", c=D)
                    bvs = bvb[:, mt * P: (mt + 1) * P].rearrange(
                        "p (g c) -> p g c", c=D)
                    nc.vector.tensor_tensor(dst, src, bvs, ALU.add)
                    nc.vector.memset(
                        Vp[:, st * 2 * HD1: (st + 1) * 2 * HD1].rearrange(
                            "p (g c) -> p g c", c=HD1)[:, :, D:HD1], 1.0)

                # scores^T + exp; the two heads alternate so their K=64
                # matmuls pack onto PE row-groups 0-63 / 64-127
                Et = {}
                for jt in range(ST):
                    for hh in range(2):
                        ho = hh * D
                        for ih in range(IH):
                            pss = ps_s.tile([P, 512], F32, tag="pss", name="pss")
                            nc.tensor.matmul(
                                pss[:],
                                lhsT=KTp[ho:ho + D, jt * P: jt * P + P],
                                rhs=QTp[ho:ho + D, ih * 512:(ih + 1) * 512],
                                start=True, stop=True)
                            et = et_pool.tile([P, 512], BF16, tag="et", name="et")
                            nc.scalar.activation(et[:], pss[:], AF.Exp,
                                                 scale=0.125)
                            Et[(hh, jt, ih)] = et

                # ctx' + output path
                out_p = pp_pool.tile([P, ST * P], F32, tag="outp", bufs=2,
                                     name="out_p")
                for hh in range(2):
                    pc = ps_c.tile([HD1, S], F32, tag="pcc", name="pcc")
                    for ih in range(IH):
                        for jt in range(ST):
                            nc.tensor.matmul(
                                pc[:, ih * 512:(ih + 1) * 512],
                                lhsT=Vp[:, jt * 2 * HD1 + hh * HD1:
                                        jt * 2 * HD1 + (hh + 1) * HD1],
                                rhs=Et[(hh, jt, ih)][:],
                                start=(jt == 0), stop=(jt == ST - 1))
                    ctxb = sm_pool.tile([HD1, S], BF16, tag="ctxb", bufs=3,
                                        name="ctxb")
                    nc.vector.tensor_copy(ctxb[:], pc[:])
                    for it in range(ST):
                        po = ps_o.tile([P, HD1], BF16, tag="po", name="po")
                        nc.tensor.transpose(
                            po[:], ctxb[:, it * P:(it + 1) * P],
                            ident[0:HD1, 0:HD1])
                        rinv = sm_pool.tile([P, 1], F32, tag="rinv", bufs=4,
                                            name="rinv")
                        nc.vector.reciprocal(rinv[:], po[:, D:HD1])
                        nc.vector.tensor_scalar_mul(
                            out_p[:, it * P + hh * D: it * P + hh * D + D],
                            po[:, 0:D], rinv[:])

                nc.sync.dma_start(
                    out_ext.rearrange("(t p) (g c) -> p t g c", p=P, c=P)[
                        :, :, mt, :],
                    out_p.rearrange("p (t c) -> p t c", c=P))

    nc.compile()
    return nc


def run(inputs, trace=False, trace_kwargs=None):
    """inputs: dict of full-shape np arrays as in reference.setup_inputs()."""
    nc = build_kernel()
    in_maps = []
    for b in range(N_CORES):
        in_maps.append({
            "from_tensor": np.ascontiguousarray(np.asarray(inputs["from_tensor"][b], dtype=np.float32)),
            "to_tensor": np.ascontiguousarray(np.asarray(inputs["to_tensor"][b], dtype=np.float32)),
            "Wq": np.asarray(inputs["Wq"], dtype=np.float32),
            "bq": np.asarray(inputs["bq"], dtype=np.float32),
            "Wk": np.asarray(inputs["Wk"], dtype=np.float32),
            "bk": np.asarray(inputs["bk"], dtype=np.float32),
            "Wv": np.asarray(inputs["Wv"], dtype=np.float32),
            "bv": np.asarray(inputs["bv"], dtype=np.float32),
        })
    res = run_bass_kernel_spmd(nc, in_maps, core_ids=list(range(N_CORES)),
                               trace=trace, **(trace_kwargs or {}))
    out = np.stack([np.asarray(res.results[b]["out"]) for b in range(N_CORES)],
                   axis=0).astype(np.float32)
    return out, res


def kernel(**inputs):
    out, _ = run(inputs, trace=False)
    return out
